# revision 21
# baseline (speedup 1.0000x reference)
"""Bayesian STDP WTA model on Trainium2 (Bass/Tile), single-core sequential kernel.

Algorithm notes:
  - The per-step recurrence (W updated every step from a sampled winner) is
    inherently sequential, and small-payload cross-core collectives have a
    ~20us floor on trn2, so the 10000-step loop runs on ONE NeuronCore.
  - Phase A computes exponential-PSP filters for all T steps as chunked
    matmuls against a 128x128 upper-triangular decay-Toeplitz matrix
    (this also transposes spikes [T,I] -> psps^T [I,T] for free), with the
    cross-chunk carry applied as a per-partition scalar_tensor_tensor.
  - Phase B is a hardware For_i loop, UNROLL steps per body. Each step:
      z = W @ psp + b    (16 K-chunk matmuls accumulating in PSUM, N=512,
                          + one K=1 matmul folding in the bias row)
      winner sampling: rowmax -> ACT exp(z-max) with fused sum -> DVE
      prefix-scan cumsum -> fused (cum < u*S) compare-and-count -> clamp,
      cast to int32, load into DVE+ACT registers for dynamic addressing.
      STDP update touches only column `idx` of W^T (and b[idx]).
  - trace output is reconstructed on the host from the returned one-hot
    spikes (exact same sequential recurrence in float32).
"""

import numpy as np

T_FULL, I_DIM, O_DIM = 10000, 2048, 512
DT = 0.001
PSP_DECAY = float(np.exp(-DT / 0.02))
OUT_DECAY = float(np.exp(-DT / 0.01))
STDP_MU = 0.1
KC = I_DIM // 128            # 16 K-chunks of the GEMV contraction
UNROLL = 8                   # phase-B steps per For_i body


def _consts():
    s = np.arange(128)
    # Ut[s, t] = decay^(t-s) for t >= s (upper-triangular Toeplitz)
    d = s[None, :] - s[:, None]
    Ut = np.where(d >= 0, np.float64(PSP_DECAY) ** np.maximum(d, 0), 0.0)
    # DVECR[p, t] = decay^(t+1), same for every partition p
    dv = np.broadcast_to(np.float64(PSP_DECAY) ** (s + 1), (128, 128))
    ident = np.eye(128)
    return (Ut.astype(np.float32), dv.astype(np.float32).copy(),
            ident.astype(np.float32))


def build_program(T):
    import concourse.bacc as bacc
    import concourse.bass as bass
    import concourse.mybir as mybir
    import concourse.tile as tile
    from concourse.bass import ds

    f32 = mybir.dt.float32
    i32 = mybir.dt.int32
    Alu = mybir.AluOpType
    Act = mybir.ActivationFunctionType
    ET = mybir.EngineType

    assert T % UNROLL == 0
    n_bodies = T // UNROLL

    nc = bacc.Bacc()
    spikes = nc.declare_dram_parameter("spikes", [T, I_DIM], f32, isOutput=False)
    u_in = nc.declare_dram_parameter("u", [1, T], f32, isOutput=False)
    W_in = nc.declare_dram_parameter("Wp", [O_DIM, I_DIM], f32, isOutput=False)
    b_in = nc.declare_dram_parameter("bp", [1, O_DIM], f32, isOutput=False)
    Ut_in = nc.declare_dram_parameter("Ut", [128, 128], f32, isOutput=False)
    dv_in = nc.declare_dram_parameter("DVECR", [128, 128], f32, isOutput=False)
    id_in = nc.declare_dram_parameter("ID128", [128, 128], f32, isOutput=False)
    zout = nc.declare_dram_parameter("z_outs", [T, O_DIM], f32, isOutput=True)

    with tile.TileContext(nc) as tc:
        with (
            tc.tile_pool(name="dram", bufs=1, space="DRAM") as dramp,
            tc.tile_pool(name="persist", bufs=1) as pp,
        ):
            psps = dramp.tile([I_DIM, T], f32)          # psps^T, [I, T]

            # ---- persistent SBUF state ----
            W_sb = pp.tile([128, KC, O_DIM], f32)       # W^T: [p, c, o] = W[o, c*128+p]
            b_row = pp.tile([1, O_DIM], f32)
            u_sb = pp.tile([1, T], f32)
            Ut_sb = pp.tile([128, 128], f32)
            dv_sb = pp.tile([128, 128], f32)
            id_sb = pp.tile([128, 128], f32)
            one11 = pp.tile([1, 1], f32)
            ones_col = pp.tile([128, 1], f32)
            zrow = pp.tile([1, O_DIM], f32)
            carry = pp.tile([128, KC], f32)

            nc.sync.dma_start(out=b_row[:], in_=b_in[:])
            nc.sync.dma_start(out=u_sb[:], in_=u_in[:])
            nc.sync.dma_start(out=Ut_sb[:], in_=Ut_in[:])
            nc.sync.dma_start(out=dv_sb[:], in_=dv_in[:])
            nc.sync.dma_start(out=id_sb[:], in_=id_in[:])
            nc.vector.memset(one11[:], 1.0)
            nc.vector.memset(ones_col[:], 1.0)
            nc.vector.memset(zrow[:], 0.0)
            nc.vector.memset(carry[:], 0.0)

            # ---- W transpose: W [512, 2048] -> W_sb [128, c, 512] ----
            with (
                tc.tile_pool(name="wload", bufs=2) as wl,
                tc.tile_pool(name="wtp", bufs=2, space="PSUM") as wtp,
            ):
                for ot in range(O_DIM // 128):
                    wrow = wl.tile([128, I_DIM], f32, tag="wrow")
                    nc.sync.dma_start(out=wrow[:], in_=W_in[ot * 128:(ot + 1) * 128, :])
                    for c in range(KC):
                        tp = wtp.tile([128, 128], f32, tag="tp")
                        nc.tensor.transpose(tp[:], wrow[:, c * 128:(c + 1) * 128], id_sb[:])
                        nc.vector.tensor_copy(W_sb[:, c, ot * 128:(ot + 1) * 128], tp[:])

            # ---- Phase A: PSPs ----
            n_full, rem = T // 128, T % 128
            chunks = [(ci * 128, 128) for ci in range(n_full)]
            if rem:
                chunks.append((n_full * 128, rem))
            with (
                tc.tile_pool(name="spk", bufs=2) as spk_p,
                tc.tile_pool(name="pab", bufs=2) as pab_p,
                tc.tile_pool(name="pa", bufs=1, space="PSUM") as pa_p,
            ):
                for (t0, tl) in chunks:
                    s_c = spk_p.tile([128, I_DIM], f32, tag="spk")
                    nc.sync.dma_start(out=s_c[:tl, :], in_=spikes[t0:t0 + tl, :])
                    for g in range(4):
                        ps = pa_p.tile([128, 512], f32, tag=f"pg{g}")
                        sb = pab_p.tile([128, 512], f32, tag=f"sb{g}")
                        for k in range(4):
                            it = g * 4 + k
                            sl = ps[:, k * 128:k * 128 + tl]
                            nc.tensor.matmul(sl, s_c[:tl, it * 128:(it + 1) * 128],
                                             Ut_sb[:tl, :tl], start=True, stop=True)
                            nc.vector.scalar_tensor_tensor(
                                sl, dv_sb[:, :tl], carry[:, it:it + 1], sl,
                                op0=Alu.mult, op1=Alu.add)
                            nc.vector.tensor_copy(carry[:, it:it + 1],
                                                  ps[:, k * 128 + tl - 1:k * 128 + tl])
                            nc.scalar.copy(sb[:, k * 128:k * 128 + tl], sl)
                            nc.sync.dma_start(
                                out=psps[it * 128:(it + 1) * 128, t0:t0 + tl],
                                in_=sb[:, k * 128:k * 128 + tl])

            # ---- Phase B: sequential loop ----
            psps_v = psps[:].rearrange("(c p) t -> p c t", p=128)
            with (
                tc.tile_pool(name="pspb", bufs=3) as pspb_p,
                tc.tile_pool(name="zo", bufs=2) as zo_p,
                tc.tile_pool(name="rows", bufs=2) as rp,
                tc.tile_pool(name="scal", bufs=2) as sp,
                tc.tile_pool(name="wup", bufs=2) as wp,
                tc.tile_pool(name="zps", bufs=2, space="PSUM") as zps_p,
            ):
                with tc.For_i(0, n_bodies, hint_engines=(ET.PE, ET.DVE, ET.Activation)) as iv:
                    s0 = iv * UNROLL
                    psp_t = pspb_p.tile([128, KC, UNROLL], f32, tag="psp")
                    nc.sync.dma_start(out=psp_t[:], in_=psps_v[:, :, ds(s0, UNROLL)])
                    zo_rows = []
                    for j in range(UNROLL):
                        zr_t = zo_p.tile([1, O_DIM], f32, tag=f"zo{j}")
                        nc.gpsimd.memset(zr_t[:], 0.0)
                        zo_rows.append(zr_t)
                    for j in range(UNROLL):
                        z_ps = zps_p.tile([1, O_DIM], f32, tag="z")
                        for c in range(KC):
                            nc.tensor.matmul(z_ps[:], psp_t[:, c, j:j + 1],
                                             W_sb[:, c, :], start=(c == 0), stop=False)
                        nc.tensor.matmul(z_ps[:], one11[:], b_row[:],
                                         start=False, stop=True)
                        negm = sp.tile([1, 1], f32, tag="negm")
                        nc.vector.tensor_reduce(negm[:], z_ps[:],
                                                axis=mybir.AxisListType.X, op=Alu.max,
                                                negate=True)
                        e_row = rp.tile([1, O_DIM], f32, tag="erow")
                        S_sb = sp.tile([1, 1], f32, tag="S")
                        nc.scalar.activation(e_row[:], z_ps[:], Act.Exp,
                                             bias=negm[:], scale=1.0, accum_out=S_sb[:])
                        cum = rp.tile([1, O_DIM], f32, tag="cum")
                        nc.vector.tensor_tensor_scan(cum[:], e_row[:], zrow[:], 0.0,
                                                     op0=Alu.add, op1=Alu.add)
                        theta = sp.tile([1, 1], f32, tag="th")
                        nc.vector.tensor_tensor(theta[:], S_sb[:],
                                                u_sb[0:1, ds(s0 + j, 1)], op=Alu.mult)
                        scr = rp.tile([1, O_DIM], f32, tag="scr")
                        cnt = sp.tile([1, 1], f32, tag="cnt")
                        nc.vector.tensor_scalar(scr[:], cum[:], theta[:], None,
                                                op0=Alu.is_lt, op1=Alu.add,
                                                accum_out=cnt[:])
                        idxi = sp.tile([1, 1], i32, tag="idxi")
                        nc.vector.tensor_scalar(idxi[:], cnt[:], float(O_DIM - 1), None,
                                                op0=Alu.min)
                        idx = nc.values_load(idxi[0:1, 0:1],
                                             engines=(ET.DVE, ET.Activation),
                                             min_val=0, max_val=O_DIM - 1,
                                             skip_runtime_bounds_check=True)
                        # ---- STDP update of column idx of W^T ----
                        wcol = W_sb[:, :, ds(idx, 1)].rearrange("p c o -> p (c o)")
                        ew = wp.tile([128, KC], f32, tag="ew")
                        nc.scalar.activation(ew[:], wcol, Act.Exp, scale=-1.0)
                        t1 = wp.tile([128, KC], f32, tag="t1")
                        nc.vector.scalar_tensor_tensor(
                            t1[:], ew[:], STDP_MU, psp_t[:, :, j],
                            op0=Alu.mult, op1=Alu.mult)
                        nc.vector.scalar_tensor_tensor(
                            wcol, t1[:], -STDP_MU, wcol,
                            op0=Alu.add, op1=Alu.add)
                        # ---- b update ----
                        bcol = b_row[0:1, ds(idx, 1)]
                        eb = sp.tile([1, 1], f32, tag="eb")
                        nc.scalar.activation(eb[:], bcol, Act.Exp, scale=-1.0)
                        dbv = sp.tile([1, 1], f32, tag="db")
                        nc.vector.tensor_scalar(dbv[:], eb[:], STDP_MU, -STDP_MU,
                                                op0=Alu.mult, op1=Alu.add)
                        nc.vector.tensor_tensor(bcol, bcol, dbv[:], op=Alu.add)
                        # ---- one-hot output ----
                        nc.vector.memset(zo_rows[j][0:1, ds(idx, 1)], 1.0)
                        nc.sync.dma_start(out=zout[ds(s0 + j, 1), :],
                                          in_=zo_rows[j][:])
    nc.finalize()
    return nc


def run_device(inputs, T, trace=False):
    from concourse.bass_utils import run_bass_kernel_spmd
    Ut, dv, ident = _consts()
    nc = build_program(T)
    in_map = {
        "spikes": np.ascontiguousarray(inputs["input_spikes"], dtype=np.float32),
        "u": np.ascontiguousarray(inputs["u_rand"], dtype=np.float32).reshape(1, T),
        "Wp": np.ascontiguousarray(inputs["W"], dtype=np.float32),
        "bp": np.ascontiguousarray(inputs["b"], dtype=np.float32).reshape(1, O_DIM),
        "Ut": Ut, "DVECR": dv, "ID128": ident,
    }
    res = run_bass_kernel_spmd(nc, [in_map], [0], trace=trace)
    if trace:
        return res.results[0]["z_outs"], res
    return res.results[0]["z_outs"]


def _trace_from_zouts(z_outs):
    dec = np.float32(OUT_DECAY)
    tr = np.zeros(z_outs.shape[1], np.float32)
    for t in range(z_outs.shape[0]):
        tr = dec * tr + z_outs[t]
    return tr


def kernel(input_spikes, u_rand, W, b):
    inputs = {"input_spikes": np.asarray(input_spikes), "u_rand": np.asarray(u_rand),
              "W": np.asarray(W), "b": np.asarray(b)}
    z_outs = np.asarray(run_device(inputs, T_FULL), dtype=np.float32)
    return z_outs, _trace_from_zouts(z_outs)


# revision 22
# speedup vs baseline: 1.0834x; 1.0834x over previous
"""Bayesian STDP WTA model on Trainium2 (Bass/Tile), single-core sequential kernel.

Algorithm notes:
  - The per-step recurrence (W updated every step from a sampled winner) is
    inherently sequential, and small-payload cross-core collectives have a
    ~20us floor on trn2, so the 10000-step loop runs on ONE NeuronCore.
  - Phase A computes exponential-PSP filters for all T steps as chunked
    matmuls against a 128x128 upper-triangular decay-Toeplitz matrix
    (this also transposes spikes [T,I] -> psps^T [I,T] for free), with the
    cross-chunk carry applied as a per-partition scalar_tensor_tensor.
  - Phase B is a hardware For_i loop, UNROLL steps per body. Each step:
      z = W @ psp + b    (16 K-chunk matmuls accumulating in PSUM, N=512,
                          + one K=1 matmul folding in the bias row)
      winner sampling: rowmax -> ACT exp(z-max) with fused sum -> DVE
      prefix-scan cumsum -> fused (cum < u*S) compare-and-count -> clamp,
      cast to int32, load into DVE+ACT registers for dynamic addressing.
      STDP update touches only column `idx` of W^T (and b[idx]).
  - trace output is reconstructed on the host from the returned one-hot
    spikes (exact same sequential recurrence in float32).
"""

import numpy as np

T_FULL, I_DIM, O_DIM = 10000, 2048, 512
DT = 0.001
PSP_DECAY = float(np.exp(-DT / 0.02))
OUT_DECAY = float(np.exp(-DT / 0.01))
STDP_MU = 0.1
KC = I_DIM // 128            # 16 K-chunks of the GEMV contraction
UNROLL = 8                   # phase-B steps per For_i body


def _consts():
    s = np.arange(128)
    # Ut[s, t] = decay^(t-s) for t >= s (upper-triangular Toeplitz)
    d = s[None, :] - s[:, None]
    Ut = np.where(d >= 0, np.float64(PSP_DECAY) ** np.maximum(d, 0), 0.0)
    # DVECR[p, t] = decay^(t+1), same for every partition p
    dv = np.broadcast_to(np.float64(PSP_DECAY) ** (s + 1), (128, 128))
    ident = np.eye(128)
    return (Ut.astype(np.float32), dv.astype(np.float32).copy(),
            ident.astype(np.float32))


def build_program(T):
    import concourse.bacc as bacc
    import concourse.bass as bass
    import concourse.mybir as mybir
    import concourse.tile as tile
    from concourse.bass import ds

    f32 = mybir.dt.float32
    i32 = mybir.dt.int32
    Alu = mybir.AluOpType
    Act = mybir.ActivationFunctionType
    ET = mybir.EngineType

    assert T % UNROLL == 0
    n_bodies = T // UNROLL

    nc = bacc.Bacc()
    spikes = nc.declare_dram_parameter("spikes", [T, I_DIM], f32, isOutput=False)
    u_in = nc.declare_dram_parameter("u", [1, T], f32, isOutput=False)
    W_in = nc.declare_dram_parameter("Wp", [O_DIM, I_DIM], f32, isOutput=False)
    b_in = nc.declare_dram_parameter("bp", [1, O_DIM], f32, isOutput=False)
    Ut_in = nc.declare_dram_parameter("Ut", [128, 128], f32, isOutput=False)
    dv_in = nc.declare_dram_parameter("DVECR", [128, 128], f32, isOutput=False)
    id_in = nc.declare_dram_parameter("ID128", [128, 128], f32, isOutput=False)
    zout = nc.declare_dram_parameter("z_outs", [T, O_DIM], f32, isOutput=True)

    with tile.TileContext(nc) as tc:
        with (
            tc.tile_pool(name="dram", bufs=1, space="DRAM") as dramp,
            tc.tile_pool(name="persist", bufs=1) as pp,
        ):
            psps = dramp.tile([I_DIM, T], f32)          # psps^T, [I, T]

            # ---- persistent SBUF state ----
            W_sb = pp.tile([128, KC, O_DIM], f32)       # W^T: [p, c, o] = W[o, c*128+p]
            b_row = pp.tile([1, O_DIM], f32)
            u_sb = pp.tile([1, T], f32)
            Ut_sb = pp.tile([128, 128], f32)
            dv_sb = pp.tile([128, 128], f32)
            id_sb = pp.tile([128, 128], f32)
            one11 = pp.tile([1, 1], f32)
            ones_col = pp.tile([128, 1], f32)
            zrow = pp.tile([1, O_DIM], f32)
            carry = pp.tile([128, KC], f32)

            nc.sync.dma_start(out=b_row[:], in_=b_in[:])
            nc.sync.dma_start(out=u_sb[:], in_=u_in[:])
            nc.sync.dma_start(out=Ut_sb[:], in_=Ut_in[:])
            nc.sync.dma_start(out=dv_sb[:], in_=dv_in[:])
            nc.sync.dma_start(out=id_sb[:], in_=id_in[:])
            nc.vector.memset(one11[:], 1.0)
            nc.vector.memset(ones_col[:], 1.0)
            nc.vector.memset(zrow[:], 0.0)
            nc.vector.memset(carry[:], 0.0)

            # ---- W transpose: W [512, 2048] -> W_sb [128, c, 512] ----
            with (
                tc.tile_pool(name="wload", bufs=2) as wl,
                tc.tile_pool(name="wtp", bufs=2, space="PSUM") as wtp,
            ):
                for ot in range(O_DIM // 128):
                    wrow = wl.tile([128, I_DIM], f32, tag="wrow")
                    nc.sync.dma_start(out=wrow[:], in_=W_in[ot * 128:(ot + 1) * 128, :])
                    for c in range(KC):
                        tp = wtp.tile([128, 128], f32, tag="tp")
                        nc.tensor.transpose(tp[:], wrow[:, c * 128:(c + 1) * 128], id_sb[:])
                        nc.vector.tensor_copy(W_sb[:, c, ot * 128:(ot + 1) * 128], tp[:])

            # ---- Phase A: PSPs ----
            n_full, rem = T // 128, T % 128
            chunks = [(ci * 128, 128) for ci in range(n_full)]
            if rem:
                chunks.append((n_full * 128, rem))
            with (
                tc.tile_pool(name="spk", bufs=2) as spk_p,
                tc.tile_pool(name="pab", bufs=2) as pab_p,
                tc.tile_pool(name="pa", bufs=1, space="PSUM") as pa_p,
            ):
                for (t0, tl) in chunks:
                    s_c = spk_p.tile([128, I_DIM], f32, tag="spk")
                    nc.sync.dma_start(out=s_c[:tl, :], in_=spikes[t0:t0 + tl, :])
                    for g in range(4):
                        ps = pa_p.tile([128, 512], f32, tag=f"pg{g}")
                        sb = pab_p.tile([128, 512], f32, tag=f"sb{g}")
                        for k in range(4):
                            it = g * 4 + k
                            sl = ps[:, k * 128:k * 128 + tl]
                            nc.tensor.matmul(sl, s_c[:tl, it * 128:(it + 1) * 128],
                                             Ut_sb[:tl, :tl], start=True, stop=True)
                            nc.vector.scalar_tensor_tensor(
                                sl, dv_sb[:, :tl], carry[:, it:it + 1], sl,
                                op0=Alu.mult, op1=Alu.add)
                            nc.vector.tensor_copy(carry[:, it:it + 1],
                                                  ps[:, k * 128 + tl - 1:k * 128 + tl])
                            nc.scalar.copy(sb[:, k * 128:k * 128 + tl], sl)
                            nc.sync.dma_start(
                                out=psps[it * 128:(it + 1) * 128, t0:t0 + tl],
                                in_=sb[:, k * 128:k * 128 + tl])

            # ---- Phase B: sequential loop ----
            psps_v = psps[:].rearrange("(c p) t -> p c t", p=128)
            with (
                tc.tile_pool(name="pspb", bufs=3) as pspb_p,
                tc.tile_pool(name="zo", bufs=2) as zo_p,
                tc.tile_pool(name="rows", bufs=2) as rp,
                tc.tile_pool(name="scal", bufs=2) as sp,
                tc.tile_pool(name="wup", bufs=2) as wp,
                tc.tile_pool(name="zps", bufs=2, space="PSUM") as zps_p,
            ):
                with tc.For_i(0, n_bodies, hint_engines=(ET.PE, ET.DVE, ET.Activation),
                              staggered_reset=True) as iv:
                    s0 = iv * UNROLL
                    psp_t = pspb_p.tile([128, KC, UNROLL], f32, tag="psp")
                    nc.sync.dma_start(out=psp_t[:], in_=psps_v[:, :, ds(s0, UNROLL)])
                    zo_rows = []
                    for j in range(UNROLL):
                        zr_t = zo_p.tile([1, O_DIM], f32, tag=f"zo{j}")
                        nc.gpsimd.memset(zr_t[:], 0.0)
                        zo_rows.append(zr_t)
                    for j in range(UNROLL):
                        z_ps = zps_p.tile([1, O_DIM], f32, tag="z")
                        for c in range(KC):
                            nc.tensor.matmul(z_ps[:], psp_t[:, c, j:j + 1],
                                             W_sb[:, c, :], start=(c == 0), stop=False)
                        nc.tensor.matmul(z_ps[:], one11[:], b_row[:],
                                         start=False, stop=True)
                        negm = sp.tile([1, 1], f32, tag="negm")
                        nc.vector.tensor_reduce(negm[:], z_ps[:],
                                                axis=mybir.AxisListType.X, op=Alu.max,
                                                negate=True)
                        e_row = rp.tile([1, O_DIM], f32, tag="erow")
                        S_sb = sp.tile([1, 1], f32, tag="S")
                        nc.scalar.activation(e_row[:], z_ps[:], Act.Exp,
                                             bias=negm[:], scale=1.0, accum_out=S_sb[:])
                        cum = rp.tile([1, O_DIM], f32, tag="cum")
                        nc.vector.tensor_tensor_scan(cum[:], e_row[:], zrow[:], 0.0,
                                                     op0=Alu.add, op1=Alu.add)
                        theta = sp.tile([1, 1], f32, tag="th")
                        nc.vector.tensor_tensor(theta[:], S_sb[:],
                                                u_sb[0:1, ds(s0 + j, 1)], op=Alu.mult)
                        scr = rp.tile([1, O_DIM], f32, tag="scr")
                        cnt = sp.tile([1, 1], f32, tag="cnt")
                        nc.vector.tensor_scalar(scr[:], cum[:], theta[:], None,
                                                op0=Alu.is_lt, op1=Alu.add,
                                                accum_out=cnt[:])
                        idxi = sp.tile([1, 1], i32, tag="idxi")
                        nc.vector.tensor_scalar(idxi[:], cnt[:], float(O_DIM - 1), None,
                                                op0=Alu.min)
                        idx = nc.values_load(idxi[0:1, 0:1],
                                             engines=(ET.DVE, ET.Activation),
                                             min_val=0, max_val=O_DIM - 1,
                                             skip_runtime_bounds_check=True)
                        # ---- STDP update of column idx of W^T ----
                        wcol = W_sb[:, :, ds(idx, 1)].rearrange("p c o -> p (c o)")
                        ew = wp.tile([128, KC], f32, tag="ew")
                        nc.scalar.activation(ew[:], wcol, Act.Exp, scale=-1.0)
                        t1 = wp.tile([128, KC], f32, tag="t1")
                        nc.vector.scalar_tensor_tensor(
                            t1[:], ew[:], STDP_MU, psp_t[:, :, j],
                            op0=Alu.mult, op1=Alu.mult)
                        nc.vector.scalar_tensor_tensor(
                            wcol, t1[:], -STDP_MU, wcol,
                            op0=Alu.add, op1=Alu.add)
                        # ---- b update ----
                        bcol = b_row[0:1, ds(idx, 1)]
                        eb = sp.tile([1, 1], f32, tag="eb")
                        nc.scalar.activation(eb[:], bcol, Act.Exp, scale=-1.0)
                        dbv = sp.tile([1, 1], f32, tag="db")
                        nc.vector.tensor_scalar(dbv[:], eb[:], STDP_MU, -STDP_MU,
                                                op0=Alu.mult, op1=Alu.add)
                        nc.vector.tensor_tensor(bcol, bcol, dbv[:], op=Alu.add)
                        # ---- one-hot output ----
                        nc.vector.memset(zo_rows[j][0:1, ds(idx, 1)], 1.0)
                        nc.sync.dma_start(out=zout[ds(s0 + j, 1), :],
                                          in_=zo_rows[j][:])
    nc.finalize()
    return nc


def run_device(inputs, T, trace=False):
    from concourse.bass_utils import run_bass_kernel_spmd
    Ut, dv, ident = _consts()
    nc = build_program(T)
    in_map = {
        "spikes": np.ascontiguousarray(inputs["input_spikes"], dtype=np.float32),
        "u": np.ascontiguousarray(inputs["u_rand"], dtype=np.float32).reshape(1, T),
        "Wp": np.ascontiguousarray(inputs["W"], dtype=np.float32),
        "bp": np.ascontiguousarray(inputs["b"], dtype=np.float32).reshape(1, O_DIM),
        "Ut": Ut, "DVECR": dv, "ID128": ident,
    }
    res = run_bass_kernel_spmd(nc, [in_map], [0], trace=trace)
    if trace:
        return res.results[0]["z_outs"], res
    return res.results[0]["z_outs"]


def _trace_from_zouts(z_outs):
    dec = np.float32(OUT_DECAY)
    tr = np.zeros(z_outs.shape[1], np.float32)
    for t in range(z_outs.shape[0]):
        tr = dec * tr + z_outs[t]
    return tr


def kernel(input_spikes, u_rand, W, b):
    inputs = {"input_spikes": np.asarray(input_spikes), "u_rand": np.asarray(u_rand),
              "W": np.asarray(W), "b": np.asarray(b)}
    z_outs = np.asarray(run_device(inputs, T_FULL), dtype=np.float32)
    return z_outs, _trace_from_zouts(z_outs)
